# revision 2
# baseline (speedup 1.0000x reference)
"""Paged-attention decode kernel for 8 Trainium2 NeuronCores — v4.

Baseline slot/bucket structure (LW-cheap PV with stationary=pt, per-slot PSUM
accumulation, on-device normalize) with a slimmed instruction stream:
  - CG=1024 gather groups -> 12 dma_gathers per rep (was 22)
  - consts in 3 DMAs: idx; qT|mask8 merged bf16; sel
  - one merged [16, 1024] output DMA (was 4)
  - per-group fused exp + mask over [128, nch*G*KVH] h-major scores (was
    per-(group,head)), denominators per (group,head) into a [32, 8] PSUM tile
    (col = head; rows c*G+g are mod-4 alignment-immune across groups), one
    sel fold matmul + one reciprocal per slot
  - PV accumulates into two [4, 512] PSUM tiles (4 heads each)
"""

import os
import sys
from contextlib import ExitStack

import numpy as np

for _p in ("/opt/trn_rl_repo", "/root/.axon_site/_ro/trn_rl_repo"):
    if os.path.isdir(_p) and _p not in sys.path:
        sys.path.insert(0, _p)

import ml_dtypes  # noqa: E402

import concourse.bass as bass  # noqa: E402
from concourse import bacc  # noqa: E402
import concourse.tile as tile  # noqa: E402
from concourse import mybir  # noqa: E402

B = 32
NUM_BLOCKS = 2048
BLOCK_SIZE = 16
KVH = 8
NH = 32
D = 128
G = NH // KVH
ROWS = NUM_BLOCKS * BLOCK_SIZE
ROW_ELEMS = KVH * D
SCALE = float(1.0 / np.sqrt(D))
N_CORES = 8
SLOTS = 4
CG = int(os.environ.get("KRN_CG", "512"))
NQ = int(os.environ.get("KRN_NQ", "2"))
KVBUFS = int(os.environ.get("KRN_KVBUFS", "8"))
SCRATCH = int(os.environ.get("KRN_SCRATCH", "16384"))
CHUNK = 128
BF16 = mybir.dt.bfloat16
F32 = mybir.dt.float32
E3M4 = mybir.dt.float8e3
V8_THRESH = 128

_prog_cache: dict = {}


def _cg_sizes(b):
    return [min(CG, b - i * CG) for i in range(-(-b // CG))]


def _layout(buckets):
    """msk8 column offsets per cg (h-major score layout, nch*G*KVH cols)."""
    offs, off = [], 0
    for b in buckets:
        for t in _cg_sizes(b):
            nch = t // CHUNK
            offs.append((off, nch))
            off += nch * G * KVH
    return offs, off


def _build_program(spec, repeat=1):
    buckets, v8 = spec
    cg_cols, MC8 = _layout(buckets)
    IDXC = sum(b // 16 for b in buckets)
    QMC = 128 + MC8
    NCGMAX = max(len(_cg_sizes(b)) for b in buckets)
    R16 = (CG // CHUNK) * G  # d16 rows per group

    nc = bacc.Bacc(num_swdge_queues=NQ, dynamic_dma_scratch_size=SCRATCH)
    kc_d = nc.declare_dram_parameter("kc", [ROWS, ROW_ELEMS], BF16, isOutput=False)
    vc_d = nc.declare_dram_parameter("vc", [ROWS, ROW_ELEMS], BF16, isOutput=False)
    vc8_d = nc.declare_dram_parameter("vc8", [ROWS, ROW_ELEMS], E3M4, isOutput=False)
    qm_d = nc.declare_dram_parameter("qm", [128, QMC], BF16, isOutput=False)
    idx_d = nc.declare_dram_parameter("idx", [128, IDXC], mybir.dt.int16, isOutput=False)
    sel_d = nc.declare_dram_parameter("sel", [64, G], F32, isOutput=False)
    out_d = nc.declare_dram_parameter("out", [SLOTS * G, KVH * D], F32, isOutput=True)

    with tile.TileContext(nc) as tc, ExitStack() as ctx:
        const = ctx.enter_context(tc.tile_pool(name="const", bufs=1))
        ktp = ctx.enter_context(tc.tile_pool(name="ktp", bufs=KVBUFS))
        vtp = ctx.enter_context(tc.tile_pool(name="vtp", bufs=KVBUFS))
        ptp = ctx.enter_context(tc.tile_pool(name="ptp", bufs=6))
        scp = ctx.enter_context(tc.tile_pool(name="scp", bufs=3, space=bass.MemorySpace.PSUM))
        oap = ctx.enter_context(tc.tile_pool(name="oap", bufs=3, space=bass.MemorySpace.PSUM))
        d16p = ctx.enter_context(tc.tile_pool(name="d16p", bufs=1, space=bass.MemorySpace.PSUM))
        rp = ctx.enter_context(tc.tile_pool(name="rp", bufs=2))
        s16p = ctx.enter_context(tc.tile_pool(name="s16p", bufs=2))
        osbp = ctx.enter_context(tc.tile_pool(name="osbp", bufs=2))

        idx = const.tile([128, IDXC], mybir.dt.int16)
        nc.sync.dma_start(idx[:], idx_d[:])
        qm = const.tile([128, QMC], BF16)
        nc.sync.dma_start(qm[:], qm_d[:])
        sel = const.tile([64, G], F32)
        nc.sync.dma_start(sel[:], sel_d[:])
        ones = const.tile([128, 1], BF16)
        nc.vector.memset(ones[:], 1.0)
        qT = qm[:, 0:128]

        for _rep in range(repeat):
            for i in range(SLOTS):
                b = buckets[i]
                sizes = _cg_sizes(b)
                base_cg = sum(len(_cg_sizes(buckets[j])) for j in range(i))
                io = sum(buckets[j] // 16 for j in range(i))
                kts, vts, pts = [], [], []
                for toks in sizes:
                    isl = idx[:, io : io + toks // 16]
                    io += toks // 16
                    kt = ktp.tile([128, KVH, toks], BF16)
                    nc.gpsimd.dma_gather(
                        kt[:], kc_d[:], isl,
                        num_idxs=toks, num_idxs_reg=toks, elem_size=ROW_ELEMS,
                        transpose=True, queue_num=0,
                    )
                    vdt, vsrc = (E3M4, vc8_d) if v8[i] else (BF16, vc_d)
                    vt = vtp.tile([128, toks // CHUNK, ROW_ELEMS], vdt)
                    nc.gpsimd.dma_gather(
                        vt[:], vsrc[:], isl,
                        num_idxs=toks, num_idxs_reg=toks, elem_size=ROW_ELEMS,
                        transpose=False, queue_num=1 % NQ,
                    )
                    kts.append(kt)
                    vts.append(vt)
                ncg = len(sizes)
                d16 = d16p.tile([R16, KVH * NCGMAX], F32)
                for cgi, toks in enumerate(sizes):
                    off, nch = cg_cols[base_cg + cgi]
                    ncols = nch * G * KVH
                    sc = scp.tile([128, ncols], F32)
                    qcol = i * 32
                    for h in range(KVH):
                        for c in range(nch):
                            nc.tensor.matmul(
                                sc[:, h * nch * G + c * G : h * nch * G + (c + 1) * G],
                                kts[cgi][:, h, c * CHUNK : (c + 1) * CHUNK],
                                qT[:, qcol + h * G : qcol + (h + 1) * G],
                                start=True, stop=True,
                            )
                    pt = ptp.tile([128, ncols], BF16)
                    nc.scalar.activation(pt[:], sc[:], mybir.ActivationFunctionType.Exp)
                    nc.vector.tensor_mul(pt[:], pt[:], qm[:, 128 + off : 128 + off + ncols])
                    pts.append(pt)
                    for h in range(KVH):
                        nc.tensor.matmul(
                            d16[0 : nch * G, cgi * KVH + h : cgi * KVH + h + 1],
                            pt[:, h * nch * G : (h + 1) * nch * G],
                            ones[:],
                            start=True, stop=True,
                            skip_group_check=True,
                        )
                s16 = s16p.tile([R16, KVH * NCGMAX], F32)
                for cgi in range(ncg):
                    rows = cg_cols[base_cg + cgi][1] * G
                    nc.vector.tensor_copy(
                        s16[0:rows, cgi * KVH : (cgi + 1) * KVH],
                        d16[0:rows, cgi * KVH : (cgi + 1) * KVH],
                    )
                dn = d16p.tile([G, KVH], F32)
                for cgi in range(ncg):
                    rows = cg_cols[base_cg + cgi][1] * G
                    nc.tensor.matmul(
                        dn[0:G, 0:KVH], sel[0:rows, :],
                        s16[0:rows, cgi * KVH : (cgi + 1) * KVH],
                        start=(cgi == 0), stop=(cgi == ncg - 1),
                        skip_group_check=True,
                    )
                r = rp.tile([G, KVH], F32)
                nc.vector.reciprocal(r[:], dn[0:G, :])
                osb = osbp.tile([G, KVH * D], F32)
                tot = b // CHUNK
                for h in range(KVH):
                    oa = oap.tile([G, D], F32)
                    nmm = 0
                    for cgi, toks in enumerate(sizes):
                        off, nch = cg_cols[base_cg + cgi]
                        for c in range(nch):
                            nc.tensor.matmul(
                                oa[0:G, 0:D],
                                pts[cgi][:, h * nch * G + c * G : h * nch * G + (c + 1) * G],
                                vts[cgi][:, c, h * D : (h + 1) * D],
                                start=(nmm == 0), stop=(nmm == tot - 1),
                                skip_group_check=True,
                            )
                            nmm += 1
                    nc.vector.tensor_scalar_mul(
                        osb[0:G, h * D : (h + 1) * D],
                        oa[0:G, 0:D],
                        r[0:G, h : h + 1],
                    )
                nc.sync.dma_start(out_d[i * G : (i + 1) * G, :], osb[:])
    nc.finalize()
    return nc


def _prep(q, k, v, k_cache, v_cache, context_lens, block_tables, slot_mapping):
    lens = np.asarray(context_lens).astype(np.int64)
    bt = np.asarray(block_tables).astype(np.int64)
    sm = np.asarray(slot_mapping).astype(np.int64)

    kc = np.ascontiguousarray(np.asarray(k_cache, np.float32)).reshape(ROWS, ROW_ELEMS).copy()
    vc = np.ascontiguousarray(np.asarray(v_cache, np.float32)).reshape(ROWS, ROW_ELEMS).copy()
    kc[sm] = np.asarray(k, np.float32).reshape(B, ROW_ELEMS)
    vc[sm] = np.asarray(v, np.float32).reshape(B, ROW_ELEMS)
    kc16 = kc.astype(ml_dtypes.bfloat16)
    vc16 = vc.astype(ml_dtypes.bfloat16)

    order = np.argsort(-lens, kind="stable")
    buckets = tuple(
        max(CHUNK, int(np.ceil(lens[order[j * N_CORES]] / CHUNK)) * CHUNK)
        for j in range(SLOTS)
    )
    v8 = tuple(bool(lens[order[(j + 1) * N_CORES - 1]] >= V8_THRESH) for j in range(SLOTS))
    vc8 = vc.astype(ml_dtypes.float8_e3m4)
    cg_cols, MC8 = _layout(buckets)
    IDXC = sum(b // 16 for b in buckets)

    qs = (np.asarray(q, np.float32)[:, 0] * SCALE).reshape(B, NH, D)
    sel = (np.arange(64)[:, None] % G == np.arange(G)[None, :]).astype(np.float32)

    in_maps = []
    for n in range(N_CORES):
        qT = np.zeros((128, 128), np.float32)
        idxs = np.zeros((16, IDXC), np.int16)
        msk8 = np.zeros((128, MC8), np.float32)
        io = 0
        cgk = 0
        for j in range(SLOTS):
            s = int(order[j * N_CORES + n])
            L = int(lens[s])
            b = buckets[j]
            cols = b // 16
            nb_used = min((L + 15) // 16, cols)
            blocks = np.zeros(cols, np.int64)
            blocks[:nb_used] = bt[s, :nb_used]
            idxs[:, io : io + cols] = (
                blocks[None, :] * BLOCK_SIZE + np.arange(16)[:, None]
            ).astype(np.int16)
            io += cols
            qT[:, j * 32 : (j + 1) * 32] = qs[s].reshape(32, D).T
            t0 = 0
            for toks in _cg_sizes(b):
                off, nch = cg_cols[cgk]
                cgk += 1
                valid = (
                    np.arange(128)[:, None] + t0 + np.arange(nch)[None, :] * CHUNK
                ) < L  # [128, nch]
                blockm = np.repeat(valid.astype(np.float32), G, axis=1)  # [128, nch*G]
                msk8[:, off : off + nch * G * KVH] = np.tile(blockm, (1, KVH))
                t0 += toks
        qm = np.concatenate([qT, msk8], axis=1).astype(ml_dtypes.bfloat16)
        in_maps.append({"kc": kc16, "vc": vc16, "vc8": vc8, "qm": qm, "idx": np.ascontiguousarray(np.tile(idxs, (8, 1))), "sel": sel})
    return (buckets, v8), order, in_maps


def _assemble(order, core_outs):
    out = np.zeros((B, 1, NH, D), np.float32)
    for n in range(N_CORES):
        o = np.asarray(core_outs[n], np.float32)  # [16, 1024]
        for j in range(SLOTS):
            s = int(order[j * N_CORES + n])
            blk = o[j * G : (j + 1) * G].reshape(G, KVH, D)  # [g, h, d]
            out[s, 0, :, :] = blk.transpose(1, 0, 2).reshape(NH, D)
    return out


def kernel(q, k, v, k_cache, v_cache, context_lens, block_tables, slot_mapping):
    from concourse.bass_utils import run_bass_kernel_spmd

    spec, order, in_maps = _prep(
        q, k, v, k_cache, v_cache, context_lens, block_tables, slot_mapping
    )
    key = ("hw", spec)
    if key not in _prog_cache:
        _prog_cache[key] = _build_program(spec)
    nc = _prog_cache[key]
    res = run_bass_kernel_spmd(nc, in_maps, list(range(N_CORES)))
    return _assemble(order, [res.results[n]["out"] for n in range(N_CORES)])

